# revision 9
# baseline (speedup 1.0000x reference)
"""MoE layer (top-2 routing, 16 experts) on 8 Trainium2 NeuronCores.

Strategy: expert-parallel. The gate (logits -> top-2 -> softmax) is computed
on the host as part of the dispatch/sharding step, replicating the reference's
jax ops so routing decisions match bit-for-bit. Tokens are gathered per
expert, experts are paired two-per-core (largest-with-smallest to balance
load), and each core runs the expert FFN (silu(x @ W1e) @ W2e) over its
gathered tokens with weights held resident in SBUF.

Device layout keeps tokens on the matmul free dimension throughout (x is
shipped transposed, [D, tokens]) so no on-chip transposes are needed:
  mm1: A^T[f, tok] += W1[d, f]^T-chunks (stationary) @ x^T[d, tok]
  silu on ScalarE, PSUM -> SBUF (bf16)
  mm2: y^T[d, tok] += W2[f, d]-chunks (stationary) @ silu(A^T)[f, tok]
All matmul operands are bfloat16: measured on HW it streams 1 moving
row/cycle like float32r but with ~11ns less per-instruction overhead
(f32r's 4-byte stationary load), and it halves every DMA stream, which
kills the DMA-starved stalls + HAM clock re-drops in the lead-in.
Accumulation stays in fp32 PSUM; end-to-end rel err ~3e-3.

The host then scatter-gathers the per-(token, slot) columns back and applies
the top-2 combine weights.
"""

import os

import ml_dtypes
import numpy as np

B, T, D, F, E = 4, 2048, 1024, 2048, 16
N_CORES = 8
P = 128
D_TILES = D // P   # 8
F_TILES = F // P   # 16
N_TOK = B * T      # 8192

_nc_cache = {}
last_results = None  # BassKernelResults of the most recent run (for test.py)


def _gate(x, Wg):
    """Top-2 routing. Uses the same jax ops as the reference so the discrete
    expert choice matches it bit-for-bit; falls back to float64 numpy."""
    h = np.asarray(x, dtype=np.float32).reshape(-1, D)
    try:
        import jax
        import jax.numpy as jnp

        logits = jnp.asarray(h) @ jnp.asarray(np.asarray(Wg, dtype=np.float32))
        scores, idx = jax.lax.top_k(logits, 2)
        probs = jax.nn.softmax(scores.astype(jnp.float32), axis=-1)
        return np.asarray(idx), np.asarray(probs, dtype=np.float32)
    except Exception:
        logits = h.astype(np.float64) @ np.asarray(Wg).astype(np.float64)
        idx = np.argsort(-logits, axis=1, kind="stable")[:, :2]
        s = np.take_along_axis(logits, idx, axis=1)
        s = s - s.max(axis=1, keepdims=True)
        p = np.exp(s)
        p /= p.sum(axis=1, keepdims=True)
        return idx.astype(np.int32), p.astype(np.float32)


def _supertiles(count):
    """Decompose a token-count capacity into supertile sizes: 512s plus one
    exact-fit remainder (multiple of 4, >= 64). bf16 matmuls run at full
    speed for any moving-dim size, so no rounding to 128/256 is needed.
    Order: largest first (its long mm1 hides the w2 startup stream), then
    ascending so the last supertile is as large as possible (opens the next
    expert's weight-slot prefetch window earlier)."""
    count = max(int(count), 64)
    m = -(-count // 4) * 4
    sizes = []
    while m > 512:
        sizes.append(512)
        m -= 512
    if m < 64:
        m = 64
    sizes.append(m)
    sizes.sort(reverse=True)
    return tuple(sizes[:1] + sorted(sizes[1:]))


def _build(st0, st1):
    """Build + compile the per-core SPMD program for supertile plans st0/st1."""
    import concourse.bacc as bacc
    import concourse.mybir as mybir
    import concourse.tile as tile

    C0, C1 = sum(st0), sum(st1)
    C = C0 + C1
    SMAX = max(max(st0), max(st1))
    bf16 = mybir.dt.bfloat16
    f32 = mybir.dt.float32

    nc = bacc.Bacc("TRN2", target_bir_lowering=False, debug=False)
    xt = nc.dram_tensor("xt", [D, C], bf16, kind="ExternalInput").ap()
    # weights arrive host-permuted so every chunk DMA is a contiguous copy:
    # w1[e, chunk, p, dt, f'], w2[e, chunk, p, ft, d']
    w1 = nc.dram_tensor("w1", [2, 8, P, D_TILES, F // 8], bf16,
                        kind="ExternalInput").ap()
    w2 = nc.dram_tensor("w2", [2, 8, P, F_TILES, D // 8], bf16,
                        kind="ExternalInput").ap()
    # bf16 output halves the store stream; host combines in fp32 (adds
    # <=2.6e-3 absmax-relative rounding on top of the ~3.3e-3 matmul error)
    out = nc.dram_tensor("out", [D, C], bf16, kind="ExternalOutput").ap()
    scratch = nc.dram_tensor("warm_scratch", [1, 1], f32).ap()

    xt_v = xt.rearrange("(dt p) c -> p dt c", p=P)    # [128, 8, C]
    out_v = out.rearrange("(dt p) c -> p dt c", p=P)

    with tile.TileContext(nc) as tc:
        with (
            tc.tile_pool(name="wpool", bufs=1) as wpool,
            tc.tile_pool(name="xpool", bufs=2) as xpool,
            tc.tile_pool(name="apool", bufs=1) as apool,
            tc.tile_pool(name="opool", bufs=6) as opool,
            tc.tile_pool(name="ps1", bufs=3, space="PSUM") as ps1p,
            tc.tile_pool(name="ps2", bufs=5, space="PSUM") as ps2p,
            tc.tile_pool(name="warm", bufs=1) as warmp,
        ):
            # Short PE warm-up on memset tiles: starts the HAM clock ramp
            # (~6us of busy time at 1.2GHz before full speed) during the
            # first weight/token DMAs. 6 matmuls (~2.6us) end before those
            # DMAs land, so the warm-up never delays real work. The
            # keep-alive store rides the SCALAR queue: on the sync queue it
            # would head-of-line-block the whole weight stream behind the
            # warm-up's completion.
            wa = warmp.tile([P, P], bf16, tag="wa")
            wb = warmp.tile([P, SMAX], bf16, tag="wb")
            nc.any.memset(wa[:], 0.0)
            nc.any.memset(wb[:], 0.0)
            pw = ps2p.tile([P, SMAX], f32, tag="ps2")
            N_WARM = 6
            for k in range(N_WARM):
                nc.tensor.matmul(
                    pw[:], wa[:], wb[:],
                    start=(k == 0), stop=(k == N_WARM - 1),
                )
            wc = warmp.tile([1, 1], f32, tag="wc")
            nc.vector.tensor_copy(wc[:], pw[0:1, 0:1])
            nc.scalar.dma_start(scratch[:], wc[:])
            # Weight SBUF layout groups chunks contiguously in the free dim so
            # Tile's range tracking sees each matmul depending only on its own
            # chunk's DMA (consumption-ordered streaming).
            W1C, W2C = 8, 8                      # w1: 2 f-tiles/chunk, w2: 1 d-tile/chunk
            F_PER = F // W1C                     # 256
            D_PER = D // W2C                     # 128
            for e, sts, base in ((0, st0, 0), (1, st1, C0)):
                # Queue split: the sync HWDGE queue carries ONLY the big weight
                # streams; outputs/silu ride the scalar engine and tokens ride
                # the gpsimd SWDGE queue, so a pending weight load (blocked on
                # the previous expert's last use of the slot) never
                # head-of-line-blocks them, and the token stream runs in
                # parallel with the weight stream instead of behind it.
                xt_first = xpool.tile([P, D_TILES, SMAX], bf16, tag="xt")
                w1_ch = [
                    wpool.tile([P, D_TILES, F_PER], bf16,
                               tag=f"w1c{i}", name=f"w1c{i}_{e}")
                    for i in range(W1C)
                ]
                w2_ch = [
                    wpool.tile([P, F_TILES, D_PER], bf16,
                               tag=f"w2c{j}", name=f"w2c{j}_{e}")
                    for j in range(W2C)
                ]
                if e == 0:
                    # Startup-critical stream, interleaved in consumption
                    # order on the fast sync HWDGE queue (the gpsimd SWDGE
                    # queue takes ~5us to start issuing): w1 chunk 0 per
                    # d-tile alternating with the token d-chunk each piece
                    # unblocks, so matmul (f0, d0) starts after ~192KB.
                    for dd in range(D_TILES):
                        nc.sync.dma_start(
                            w1_ch[0][:, dd:dd + 1, :],
                            w1[e, 0, :, dd:dd + 1, :],
                        )
                        nc.sync.dma_start(
                            xt_first[:, dd, :sts[0]],
                            xt_v[:, dd, base:base + sts[0]],
                        )
                else:
                    # expert 1's first tokens have a pending buffer-recycle
                    # WAR; keep them off the sync/scalar queues so they can't
                    # head-of-line-block the weight stream or output DMAs.
                    nc.sync.dma_start(w1_ch[0][:], w1[e, 0])
                    for dd in range(D_TILES):
                        nc.gpsimd.dma_start(
                            xt_first[:, dd, :sts[0]],
                            xt_v[:, dd, base:base + sts[0]],
                        )
                for i in range(1, W1C):
                    nc.sync.dma_start(w1_ch[i][:], w1[e, i])
                for j in range(W2C):
                    nc.sync.dma_start(w2_ch[j][:], w2[e, j])
                off = base
                xt_next = xt_first
                for si, S in enumerate(sts):
                    xt_t = xt_next
                    at = apool.tile([P, F_TILES, SMAX], bf16, tag="at")
                    for f in range(F_TILES):
                        ps = ps1p.tile([P, SMAX], f32, tag="ps1")
                        for d in range(D_TILES):
                            nc.tensor.matmul(
                                ps[:, :S],
                                w1_ch[f // 2][:, d, (f % 2) * P:(f % 2 + 1) * P],
                                xt_t[:, d, :S],
                                start=(d == 0),
                                stop=(d == D_TILES - 1),
                            )
                        nc.scalar.activation(
                            at[:, f, :S], ps[:, :S],
                            mybir.ActivationFunctionType.Silu,
                        )
                        if f == 7 and si + 1 < len(sts):
                            # prefetch the next supertile's tokens mid-stream:
                            # enqueued here, its buffer-recycle WAR is already
                            # resolved, so it can't head-of-line-block the
                            # output DMAs behind it on this queue.
                            S2 = sts[si + 1]
                            o2 = off + S
                            xt_next = xpool.tile(
                                [P, D_TILES, SMAX], bf16, tag="xt"
                            )
                            nc.gpsimd.dma_start(
                                xt_next[:, :, :S2], xt_v[:, :, o2:o2 + S2]
                            )
                    for d in range(D_TILES):
                        ps = ps2p.tile([P, SMAX], f32, tag="ps2")
                        for f in range(F_TILES):
                            nc.tensor.matmul(
                                ps[:, :S],
                                w2_ch[d][:, f],
                                at[:, f, :S],
                                start=(f == 0),
                                stop=(f == F_TILES - 1),
                            )
                        ot = opool.tile([P, SMAX], bf16, tag="ot")
                        nc.vector.tensor_copy(ot[:, :S], ps[:, :S])
                        nc.scalar.dma_start(out_v[:, d, off:off + S], ot[:, :S])
                    off += S
    nc.compile()
    return nc


def kernel(x, Wg, W1, W2):
    global last_results
    import concourse.bass_utils as bass_utils

    x = np.asarray(x, dtype=np.float32)
    W1 = np.asarray(W1, dtype=np.float32)
    W2 = np.asarray(W2, dtype=np.float32)

    idx, probs = _gate(x, Wg)
    h = x.reshape(-1, D)

    counts = np.bincount(idx.ravel(), minlength=E)
    order = np.argsort(-counts, kind="stable")
    pairs = [(int(order[i]), int(order[2 * N_CORES - 1 - i])) for i in range(N_CORES)]
    cap0 = int(counts[[p[0] for p in pairs]].max())
    cap1 = int(counts[[p[1] for p in pairs]].max())
    st0, st1 = _supertiles(cap0), _supertiles(cap1)
    C0, C1 = sum(st0), sum(st1)
    C = C0 + C1

    key = (st0, st1)
    nc = _nc_cache.get(key)
    if nc is None:
        nc = _build(st0, st1)
        _nc_cache[key] = nc

    bf16 = ml_dtypes.bfloat16
    pos = np.empty((N_TOK, 2), np.int64)
    in_maps = []
    for c, (e0, e1) in enumerate(pairs):
        ids = np.zeros(C, np.int64)
        for e, off in ((e0, 0), (e1, C0)):
            tok = np.nonzero((idx[:, 0] == e) | (idx[:, 1] == e))[0]
            ids[off:off + len(tok)] = tok
            first = idx[tok, 0] == e
            gcol = c * C + off + np.arange(len(tok))
            pos[tok[first], 0] = gcol[first]
            pos[tok[~first], 1] = gcol[~first]
        # permute weights to [e, chunk, p, tiles, cols] so each chunk DMA is
        # one contiguous copy (see _build)
        w1c = (
            W1[[e0, e1]]
            .reshape(2, D_TILES, P, 8, F // 8)
            .transpose(0, 3, 2, 1, 4)
        )
        w2c = (
            W2[[e0, e1]]
            .reshape(2, F_TILES, P, 8, D // 8)
            .transpose(0, 3, 2, 1, 4)
        )
        in_maps.append({
            "xt": np.ascontiguousarray(h[ids].T).astype(bf16),
            "w1": np.ascontiguousarray(w1c).astype(bf16),
            "w2": np.ascontiguousarray(w2c).astype(bf16),
        })

    trace = os.environ.get("MOE_TRACE") == "1"
    kwargs = {}
    if trace:
        kwargs = {"trace": True, "trace_cores": list(range(N_CORES))}
    res = bass_utils.run_bass_kernel_spmd(
        nc, in_maps, core_ids=list(range(N_CORES)), **kwargs
    )
    last_results = res

    out_all = np.concatenate(
        [np.asarray(r["out"]).astype(np.float32) for r in res.results], axis=1
    )  # [D, 8*C]
    y = out_all[:, pos[:, 0]] * probs[:, 0] + out_all[:, pos[:, 1]] * probs[:, 1]
    return np.ascontiguousarray(y.T).reshape(B, T, D).astype(np.float32)


# revision 11
# speedup vs baseline: 1.0002x; 1.0002x over previous
"""MoE layer (top-2 routing, 16 experts) on 8 Trainium2 NeuronCores.

Strategy: expert-parallel. The gate (logits -> top-2 -> softmax) is computed
on the host as part of the dispatch/sharding step, replicating the reference's
jax ops so routing decisions match bit-for-bit. Tokens are gathered per
expert, experts are paired two-per-core (largest-with-smallest to balance
load), and each core runs the expert FFN (silu(x @ W1e) @ W2e) over its
gathered tokens with weights held resident in SBUF.

Device layout keeps tokens on the matmul free dimension throughout (x is
shipped transposed, [D, tokens]) so no on-chip transposes are needed:
  mm1: A^T[f, tok] += W1[d, f]^T-chunks (stationary) @ x^T[d, tok]
  silu on ScalarE, PSUM -> SBUF (bf16)
  mm2: y^T[d, tok] += W2[f, d]-chunks (stationary) @ silu(A^T)[f, tok]
All matmul operands are bfloat16: measured on HW it streams 1 moving
row/cycle like float32r but with ~11ns less per-instruction overhead
(f32r's 4-byte stationary load), and it halves every DMA stream, which
kills the DMA-starved stalls + HAM clock re-drops in the lead-in.
Accumulation stays in fp32 PSUM; end-to-end rel err ~3e-3.

The host then scatter-gathers the per-(token, slot) columns back and applies
the top-2 combine weights.
"""

import os

import ml_dtypes
import numpy as np

B, T, D, F, E = 4, 2048, 1024, 2048, 16
N_CORES = 8
P = 128
D_TILES = D // P   # 8
F_TILES = F // P   # 16
N_TOK = B * T      # 8192

_nc_cache = {}
last_results = None  # BassKernelResults of the most recent run (for test.py)


def _gate(x, Wg):
    """Top-2 routing. Uses the same jax ops as the reference so the discrete
    expert choice matches it bit-for-bit; falls back to float64 numpy."""
    h = np.asarray(x, dtype=np.float32).reshape(-1, D)
    try:
        import jax
        import jax.numpy as jnp

        logits = jnp.asarray(h) @ jnp.asarray(np.asarray(Wg, dtype=np.float32))
        scores, idx = jax.lax.top_k(logits, 2)
        probs = jax.nn.softmax(scores.astype(jnp.float32), axis=-1)
        return np.asarray(idx), np.asarray(probs, dtype=np.float32)
    except Exception:
        logits = h.astype(np.float64) @ np.asarray(Wg).astype(np.float64)
        idx = np.argsort(-logits, axis=1, kind="stable")[:, :2]
        s = np.take_along_axis(logits, idx, axis=1)
        s = s - s.max(axis=1, keepdims=True)
        p = np.exp(s)
        p /= p.sum(axis=1, keepdims=True)
        return idx.astype(np.int32), p.astype(np.float32)


def _supertiles(count):
    """Decompose a token-count capacity into supertile sizes: 512s plus one
    exact-fit remainder (multiple of 4, >= 64). bf16 matmuls run at full
    speed for any moving-dim size, so no rounding to 128/256 is needed.
    Order: largest first (its long mm1 hides the w2 startup stream), then
    ascending so the last supertile is as large as possible (opens the next
    expert's weight-slot prefetch window earlier)."""
    count = max(int(count), 64)
    m = -(-count // 4) * 4
    sizes = []
    while m > 512:
        sizes.append(512)
        m -= 512
    if m < 64:
        m = 64
    sizes.append(m)
    sizes.sort(reverse=True)
    return tuple(sizes[:1] + sorted(sizes[1:]))


def _build(st0, st1):
    """Build + compile the per-core SPMD program for supertile plans st0/st1."""
    import concourse.bacc as bacc
    import concourse.mybir as mybir
    import concourse.tile as tile

    C0, C1 = sum(st0), sum(st1)
    C = C0 + C1
    SMAX = max(max(st0), max(st1))
    bf16 = mybir.dt.bfloat16
    f32 = mybir.dt.float32

    nc = bacc.Bacc("TRN2", target_bir_lowering=False, debug=False)
    xt = nc.dram_tensor("xt", [D, C], bf16, kind="ExternalInput").ap()
    # weights arrive host-permuted so every chunk DMA is a contiguous copy:
    # w1[e, chunk, p, dt, f'], w2[e, chunk, p, ft, d']
    w1 = nc.dram_tensor("w1", [2, 8, P, D_TILES, F // 8], bf16,
                        kind="ExternalInput").ap()
    w2 = nc.dram_tensor("w2", [2, 8, P, F_TILES, D // 8], bf16,
                        kind="ExternalInput").ap()
    # bf16 output halves the store stream; host combines in fp32 (adds
    # <=2.6e-3 absmax-relative rounding on top of the ~3.3e-3 matmul error)
    out = nc.dram_tensor("out", [D, C], bf16, kind="ExternalOutput").ap()

    xt_v = xt.rearrange("(dt p) c -> p dt c", p=P)    # [128, 8, C]
    out_v = out.rearrange("(dt p) c -> p dt c", p=P)

    with tile.TileContext(nc) as tc:
        with (
            tc.tile_pool(name="wpool", bufs=1) as wpool,
            tc.tile_pool(name="xpool", bufs=2) as xpool,
            tc.tile_pool(name="apool", bufs=1) as apool,
            tc.tile_pool(name="opool", bufs=6) as opool,
            tc.tile_pool(name="ps1", bufs=3, space="PSUM") as ps1p,
            tc.tile_pool(name="ps2", bufs=5, space="PSUM") as ps2p,
        ):
            # No PE warm-up block: the HAM clock ramp (~6us of busy time at
            # 1.2GHz before full speed) progresses the same whether the PE
            # runs dummy or real matmuls, so ramping on real work is free;
            # a measured warm-up variant only added its own busy time.
            # Weight SBUF layout groups chunks contiguously in the free dim so
            # Tile's range tracking sees each matmul depending only on its own
            # chunk's DMA (consumption-ordered streaming).
            W1C, W2C = 8, 8                      # w1: 2 f-tiles/chunk, w2: 1 d-tile/chunk
            F_PER = F // W1C                     # 256
            D_PER = D // W2C                     # 128
            for e, sts, base in ((0, st0, 0), (1, st1, C0)):
                # Queue split: the sync HWDGE queue carries ONLY the big weight
                # streams; outputs/silu ride the scalar engine and tokens ride
                # the gpsimd SWDGE queue, so a pending weight load (blocked on
                # the previous expert's last use of the slot) never
                # head-of-line-blocks them, and the token stream runs in
                # parallel with the weight stream instead of behind it.
                xt_first = xpool.tile([P, D_TILES, SMAX], bf16, tag="xt")
                w1_ch = [
                    wpool.tile([P, D_TILES, F_PER], bf16,
                               tag=f"w1c{i}", name=f"w1c{i}_{e}")
                    for i in range(W1C)
                ]
                w2_ch = [
                    wpool.tile([P, F_TILES, D_PER], bf16,
                               tag=f"w2c{j}", name=f"w2c{j}_{e}")
                    for j in range(W2C)
                ]
                if e == 0:
                    # Startup-critical stream, interleaved in consumption
                    # order on the fast sync HWDGE queue (the gpsimd SWDGE
                    # queue takes ~5us to start issuing): w1 chunk 0 per
                    # d-tile alternating with the token d-chunk each piece
                    # unblocks, so matmul (f0, d0) starts after ~192KB.
                    for dd in range(D_TILES):
                        nc.sync.dma_start(
                            w1_ch[0][:, dd:dd + 1, :],
                            w1[e, 0, :, dd:dd + 1, :],
                        )
                        nc.sync.dma_start(
                            xt_first[:, dd, :sts[0]],
                            xt_v[:, dd, base:base + sts[0]],
                        )
                else:
                    # expert 1's first tokens have a pending buffer-recycle
                    # WAR; keep them off the sync/scalar queues so they can't
                    # head-of-line-block the weight stream or output DMAs.
                    nc.sync.dma_start(w1_ch[0][:], w1[e, 0])
                    for dd in range(D_TILES):
                        nc.gpsimd.dma_start(
                            xt_first[:, dd, :sts[0]],
                            xt_v[:, dd, base:base + sts[0]],
                        )
                for i in range(1, W1C):
                    nc.sync.dma_start(w1_ch[i][:], w1[e, i])
                for j in range(W2C):
                    nc.sync.dma_start(w2_ch[j][:], w2[e, j])
                off = base
                xt_next = xt_first
                for si, S in enumerate(sts):
                    xt_t = xt_next
                    at = apool.tile([P, F_TILES, SMAX], bf16, tag="at")
                    for f in range(F_TILES):
                        ps = ps1p.tile([P, SMAX], f32, tag="ps1")
                        for d in range(D_TILES):
                            nc.tensor.matmul(
                                ps[:, :S],
                                w1_ch[f // 2][:, d, (f % 2) * P:(f % 2 + 1) * P],
                                xt_t[:, d, :S],
                                start=(d == 0),
                                stop=(d == D_TILES - 1),
                            )
                        nc.scalar.activation(
                            at[:, f, :S], ps[:, :S],
                            mybir.ActivationFunctionType.Silu,
                        )
                        if f == 7 and si + 1 < len(sts):
                            # prefetch the next supertile's tokens mid-stream:
                            # enqueued here, its buffer-recycle WAR is already
                            # resolved, so it can't head-of-line-block the
                            # output DMAs behind it on this queue.
                            S2 = sts[si + 1]
                            o2 = off + S
                            xt_next = xpool.tile(
                                [P, D_TILES, SMAX], bf16, tag="xt"
                            )
                            nc.gpsimd.dma_start(
                                xt_next[:, :, :S2], xt_v[:, :, o2:o2 + S2]
                            )
                    for d in range(D_TILES):
                        ps = ps2p.tile([P, SMAX], f32, tag="ps2")
                        for f in range(F_TILES):
                            nc.tensor.matmul(
                                ps[:, :S],
                                w2_ch[d][:, f],
                                at[:, f, :S],
                                start=(f == 0),
                                stop=(f == F_TILES - 1),
                            )
                        ot = opool.tile([P, SMAX], bf16, tag="ot")
                        nc.vector.tensor_copy(ot[:, :S], ps[:, :S])
                        nc.scalar.dma_start(out_v[:, d, off:off + S], ot[:, :S])
                    off += S
    nc.compile()
    return nc


def kernel(x, Wg, W1, W2):
    global last_results
    import concourse.bass_utils as bass_utils

    x = np.asarray(x, dtype=np.float32)
    W1 = np.asarray(W1, dtype=np.float32)
    W2 = np.asarray(W2, dtype=np.float32)

    idx, probs = _gate(x, Wg)
    h = x.reshape(-1, D)

    counts = np.bincount(idx.ravel(), minlength=E)
    order = np.argsort(-counts, kind="stable")
    pairs = [(int(order[i]), int(order[2 * N_CORES - 1 - i])) for i in range(N_CORES)]
    cap0 = int(counts[[p[0] for p in pairs]].max())
    cap1 = int(counts[[p[1] for p in pairs]].max())
    st0, st1 = _supertiles(cap0), _supertiles(cap1)
    C0, C1 = sum(st0), sum(st1)
    C = C0 + C1

    key = (st0, st1)
    nc = _nc_cache.get(key)
    if nc is None:
        nc = _build(st0, st1)
        _nc_cache[key] = nc

    bf16 = ml_dtypes.bfloat16
    pos = np.empty((N_TOK, 2), np.int64)
    in_maps = []
    for c, (e0, e1) in enumerate(pairs):
        ids = np.zeros(C, np.int64)
        for e, off in ((e0, 0), (e1, C0)):
            tok = np.nonzero((idx[:, 0] == e) | (idx[:, 1] == e))[0]
            ids[off:off + len(tok)] = tok
            first = idx[tok, 0] == e
            gcol = c * C + off + np.arange(len(tok))
            pos[tok[first], 0] = gcol[first]
            pos[tok[~first], 1] = gcol[~first]
        # permute weights to [e, chunk, p, tiles, cols] so each chunk DMA is
        # one contiguous copy (see _build)
        w1c = (
            W1[[e0, e1]]
            .reshape(2, D_TILES, P, 8, F // 8)
            .transpose(0, 3, 2, 1, 4)
        )
        w2c = (
            W2[[e0, e1]]
            .reshape(2, F_TILES, P, 8, D // 8)
            .transpose(0, 3, 2, 1, 4)
        )
        in_maps.append({
            "xt": np.ascontiguousarray(h[ids].T).astype(bf16),
            "w1": np.ascontiguousarray(w1c).astype(bf16),
            "w2": np.ascontiguousarray(w2c).astype(bf16),
        })

    trace = os.environ.get("MOE_TRACE") == "1"
    kwargs = {}
    if trace:
        kwargs = {"trace": True, "trace_cores": list(range(N_CORES))}
    res = bass_utils.run_bass_kernel_spmd(
        nc, in_maps, core_ids=list(range(N_CORES)), **kwargs
    )
    last_results = res

    out_all = np.concatenate(
        [np.asarray(r["out"]).astype(np.float32) for r in res.results], axis=1
    )  # [D, 8*C]
    y = out_all[:, pos[:, 0]] * probs[:, 0] + out_all[:, pos[:, 1]] * probs[:, 1]
    return np.ascontiguousarray(y.T).reshape(B, T, D).astype(np.float32)


# revision 14
# speedup vs baseline: 1.0089x; 1.0087x over previous
"""MoE layer (top-2 routing, 16 experts) on 8 Trainium2 NeuronCores.

Strategy: expert-parallel. The gate (logits -> top-2 -> softmax) is computed
on the host as part of the dispatch/sharding step, replicating the reference's
jax ops so routing decisions match bit-for-bit. Tokens are gathered per
expert, experts are paired two-per-core (largest-with-smallest to balance
load), and each core runs the expert FFN (silu(x @ W1e) @ W2e) over its
gathered tokens with weights held resident in SBUF.

Device layout keeps tokens on the matmul free dimension throughout (x is
shipped transposed, [D, tokens]) so no on-chip transposes are needed:
  mm1: A^T[f, tok] += W1[d, f]^T-chunks (stationary) @ x^T[d, tok]
  silu on ScalarE, PSUM -> SBUF (bf16)
  mm2: y^T[d, tok] += W2[f, d]-chunks (stationary) @ silu(A^T)[f, tok]
All matmul operands are bfloat16: measured on HW it streams 1 moving
row/cycle like float32r but with ~11ns less per-instruction overhead
(f32r's 4-byte stationary load), and it halves every DMA stream, which
kills the DMA-starved stalls + HAM clock re-drops in the lead-in.
Accumulation stays in fp32 PSUM; end-to-end rel err ~3e-3.

The host then scatter-gathers the per-(token, slot) columns back and applies
the top-2 combine weights.
"""

import os

import ml_dtypes
import numpy as np

B, T, D, F, E = 4, 2048, 1024, 2048, 16
N_CORES = 8
P = 128
D_TILES = D // P   # 8
F_TILES = F // P   # 16
N_TOK = B * T      # 8192

_nc_cache = {}
last_results = None  # BassKernelResults of the most recent run (for test.py)


def _gate(x, Wg):
    """Top-2 routing. Uses the same jax ops as the reference so the discrete
    expert choice matches it bit-for-bit; falls back to float64 numpy."""
    h = np.asarray(x, dtype=np.float32).reshape(-1, D)
    try:
        import jax
        import jax.numpy as jnp

        logits = jnp.asarray(h) @ jnp.asarray(np.asarray(Wg, dtype=np.float32))
        scores, idx = jax.lax.top_k(logits, 2)
        probs = jax.nn.softmax(scores.astype(jnp.float32), axis=-1)
        return np.asarray(idx), np.asarray(probs, dtype=np.float32)
    except Exception:
        logits = h.astype(np.float64) @ np.asarray(Wg).astype(np.float64)
        idx = np.argsort(-logits, axis=1, kind="stable")[:, :2]
        s = np.take_along_axis(logits, idx, axis=1)
        s = s - s.max(axis=1, keepdims=True)
        p = np.exp(s)
        p /= p.sum(axis=1, keepdims=True)
        return idx.astype(np.int32), p.astype(np.float32)


def _supertiles(count):
    """Decompose a token-count capacity into supertile sizes: 512s plus one
    exact-fit remainder (multiple of 4, >= 64). bf16 matmuls run at full
    speed for any moving-dim size, so no rounding to 128/256 is needed.
    Order: largest first (its long mm1 hides the w2 startup stream), then
    ascending so the last supertile is as large as possible (opens the next
    expert's weight-slot prefetch window earlier)."""
    count = max(int(count), 64)
    m = -(-count // 4) * 4
    sizes = []
    while m > 512:
        sizes.append(512)
        m -= 512
    if m < 64:
        m = 64
    sizes.append(m)
    sizes.sort(reverse=True)
    return tuple(sizes[:1] + sorted(sizes[1:]))


def _build(st0, st1):
    """Build + compile the per-core SPMD program for supertile plans st0/st1."""
    import concourse.bacc as bacc
    import concourse.mybir as mybir
    import concourse.tile as tile

    C0, C1 = sum(st0), sum(st1)
    C = C0 + C1
    SMAX = max(max(st0), max(st1))
    bf16 = mybir.dt.bfloat16
    f32 = mybir.dt.float32

    nc = bacc.Bacc("TRN2", target_bir_lowering=False, debug=False)
    xt = nc.dram_tensor("xt", [D, C], bf16, kind="ExternalInput").ap()
    # First-supertile tokens, host-packed as [e, grp, p, 4dt, S] so each DMA
    # moves 4KB-contiguous per-partition runs. The startup stream is DMA
    # DESCRIPTOR-RATE limited (~80ns/descriptor/engine regardless of size),
    # not HBM-limited, so descriptor size sets the lead-in time.
    xts0 = nc.dram_tensor("xts0", [2, 2, P, 4, SMAX], bf16,
                          kind="ExternalInput").ap()
    # weights arrive host-permuted so every chunk DMA is a contiguous copy:
    # w1[e, chunk, p, dt, f'], w2[e, chunk, p, ft, d']
    w1 = nc.dram_tensor("w1", [2, 8, P, D_TILES, F // 8], bf16,
                        kind="ExternalInput").ap()
    w2 = nc.dram_tensor("w2", [2, 8, P, F_TILES, D // 8], bf16,
                        kind="ExternalInput").ap()
    # bf16 output halves the store stream; host combines in fp32 (adds
    # <=2.6e-3 absmax-relative rounding on top of the ~3.3e-3 matmul error)
    out = nc.dram_tensor("out", [D, C], bf16, kind="ExternalOutput").ap()

    xt_v = xt.rearrange("(dt p) c -> p dt c", p=P)    # [128, 8, C]
    out_v = out.rearrange("(dt p) c -> p dt c", p=P)

    with tile.TileContext(nc) as tc:
        with (
            tc.tile_pool(name="wpool", bufs=1) as wpool,
            tc.tile_pool(name="xpool", bufs=2) as xpool,
            tc.tile_pool(name="apool", bufs=1) as apool,
            tc.tile_pool(name="opool", bufs=6) as opool,
            tc.tile_pool(name="ps1", bufs=3, space="PSUM") as ps1p,
            tc.tile_pool(name="ps2", bufs=5, space="PSUM") as ps2p,
        ):
            # No PE warm-up block: the HAM clock ramp (~6us of busy time at
            # 1.2GHz before full speed) progresses the same whether the PE
            # runs dummy or real matmuls, so ramping on real work is free;
            # a measured warm-up variant only added its own busy time.
            # Weight SBUF layout groups chunks contiguously in the free dim so
            # Tile's range tracking sees each matmul depending only on its own
            # chunk's DMA (consumption-ordered streaming).
            W1C, W2C = 8, 8                      # w1: 2 f-tiles/chunk, w2: 1 d-tile/chunk
            F_PER = F // W1C                     # 256
            D_PER = D // W2C                     # 128
            for e, sts, base in ((0, st0, 0), (1, st1, C0)):
                # Queue split: the sync HWDGE queue carries ONLY the big weight
                # streams; outputs/silu ride the scalar engine and tokens ride
                # the gpsimd SWDGE queue, so a pending weight load (blocked on
                # the previous expert's last use of the slot) never
                # head-of-line-blocks them, and the token stream runs in
                # parallel with the weight stream instead of behind it.
                xt_first = xpool.tile([P, D_TILES, SMAX], bf16, tag="xt")
                w1_ch = [
                    wpool.tile([P, D_TILES, F_PER], bf16,
                               tag=f"w1c{i}", name=f"w1c{i}_{e}")
                    for i in range(W1C)
                ]
                w2_ch = [
                    wpool.tile([P, F_TILES, D_PER], bf16,
                               tag=f"w2c{j}", name=f"w2c{j}_{e}")
                    for j in range(W2C)
                ]
                if e == 0:
                    # Startup-critical stream, interleaved in consumption
                    # order on the fast sync HWDGE queue (the gpsimd SWDGE
                    # queue takes ~5us to start issuing). Half-chunk w1
                    # pieces (2KB runs) alternate with 4-dtile token groups
                    # (4KB runs) to keep descriptor counts minimal.
                    for i in range(2):
                        nc.sync.dma_start(
                            w1_ch[0][:, 4 * i:4 * i + 4, :],
                            w1[e, 0, :, 4 * i:4 * i + 4, :],
                        )
                        nc.sync.dma_start(
                            xt_first[:, 4 * i:4 * i + 4, :sts[0]],
                            xts0[e, i, :, :, :sts[0]],
                        )
                else:
                    # expert 1's first tokens have a pending buffer-recycle
                    # WAR; keep them off the sync/scalar queues so they can't
                    # head-of-line-block the weight stream or output DMAs.
                    nc.sync.dma_start(w1_ch[0][:], w1[e, 0])
                    for i in range(2):
                        nc.gpsimd.dma_start(
                            xt_first[:, 4 * i:4 * i + 4, :sts[0]],
                            xts0[e, i, :, :, :sts[0]],
                        )
                for i in range(1, W1C):
                    nc.sync.dma_start(w1_ch[i][:], w1[e, i])
                for j in range(W2C):
                    nc.sync.dma_start(w2_ch[j][:], w2[e, j])
                off = base
                xt_next = xt_first
                for si, S in enumerate(sts):
                    xt_t = xt_next
                    at = apool.tile([P, F_TILES, SMAX], bf16, tag="at")
                    for f in range(F_TILES):
                        ps = ps1p.tile([P, SMAX], f32, tag="ps1")
                        for d in range(D_TILES):
                            nc.tensor.matmul(
                                ps[:, :S],
                                w1_ch[f // 2][:, d, (f % 2) * P:(f % 2 + 1) * P],
                                xt_t[:, d, :S],
                                start=(d == 0),
                                stop=(d == D_TILES - 1),
                            )
                        nc.scalar.activation(
                            at[:, f, :S], ps[:, :S],
                            mybir.ActivationFunctionType.Silu,
                        )
                        if f == 7 and si + 1 < len(sts):
                            # prefetch the next supertile's tokens mid-stream:
                            # enqueued here, its buffer-recycle WAR is already
                            # resolved, so it can't head-of-line-block the
                            # output DMAs behind it on this queue.
                            S2 = sts[si + 1]
                            o2 = off + S
                            xt_next = xpool.tile(
                                [P, D_TILES, SMAX], bf16, tag="xt"
                            )
                            nc.gpsimd.dma_start(
                                xt_next[:, :, :S2], xt_v[:, :, o2:o2 + S2]
                            )
                    for d in range(D_TILES):
                        ps = ps2p.tile([P, SMAX], f32, tag="ps2")
                        for f in range(F_TILES):
                            nc.tensor.matmul(
                                ps[:, :S],
                                w2_ch[d][:, f],
                                at[:, f, :S],
                                start=(f == 0),
                                stop=(f == F_TILES - 1),
                            )
                        ot = opool.tile([P, SMAX], bf16, tag="ot")
                        nc.vector.tensor_copy(ot[:, :S], ps[:, :S])
                        nc.scalar.dma_start(out_v[:, d, off:off + S], ot[:, :S])
                    off += S
    nc.compile()
    return nc


def kernel(x, Wg, W1, W2):
    global last_results
    import concourse.bass_utils as bass_utils

    x = np.asarray(x, dtype=np.float32)
    W1 = np.asarray(W1, dtype=np.float32)
    W2 = np.asarray(W2, dtype=np.float32)

    idx, probs = _gate(x, Wg)
    h = x.reshape(-1, D)

    counts = np.bincount(idx.ravel(), minlength=E)
    order = np.argsort(-counts, kind="stable")
    pairs = [(int(order[i]), int(order[2 * N_CORES - 1 - i])) for i in range(N_CORES)]
    cap0 = int(counts[[p[0] for p in pairs]].max())
    cap1 = int(counts[[p[1] for p in pairs]].max())
    st0, st1 = _supertiles(cap0), _supertiles(cap1)
    C0, C1 = sum(st0), sum(st1)
    C = C0 + C1

    key = (st0, st1)
    nc = _nc_cache.get(key)
    if nc is None:
        nc = _build(st0, st1)
        _nc_cache[key] = nc

    bf16 = ml_dtypes.bfloat16
    pos = np.empty((N_TOK, 2), np.int64)
    in_maps = []
    for c, (e0, e1) in enumerate(pairs):
        ids = np.zeros(C, np.int64)
        for e, off in ((e0, 0), (e1, C0)):
            tok = np.nonzero((idx[:, 0] == e) | (idx[:, 1] == e))[0]
            ids[off:off + len(tok)] = tok
            first = idx[tok, 0] == e
            gcol = c * C + off + np.arange(len(tok))
            pos[tok[first], 0] = gcol[first]
            pos[tok[~first], 1] = gcol[~first]
        # permute weights to [e, chunk, p, tiles, cols] so each chunk DMA is
        # one contiguous copy (see _build)
        w1c = (
            W1[[e0, e1]]
            .reshape(2, D_TILES, P, 8, F // 8)
            .transpose(0, 3, 2, 1, 4)
        )
        w2c = (
            W2[[e0, e1]]
            .reshape(2, F_TILES, P, 8, D // 8)
            .transpose(0, 3, 2, 1, 4)
        )
        # first-supertile token blocks packed for 4KB-contiguous DMA runs:
        # [e, grp, p, 4dt, S] with the d axis split (grp, 4dt, p)
        SMAX = max(max(st0), max(st1))
        xts0 = np.zeros((2, 2, P, 4, SMAX), dtype=np.float32)
        for e_i, (off, sts) in enumerate(((0, st0), (C0, st1))):
            s0 = sts[0]
            blk = h[ids[off:off + s0]].T           # [D, s0]
            blk = blk.reshape(2, 4, P, s0).transpose(0, 2, 1, 3)
            xts0[e_i, :, :, :, :s0] = blk
        in_maps.append({
            "xt": np.ascontiguousarray(h[ids].T).astype(bf16),
            "xts0": xts0.astype(bf16),
            "w1": np.ascontiguousarray(w1c).astype(bf16),
            "w2": np.ascontiguousarray(w2c).astype(bf16),
        })

    trace = os.environ.get("MOE_TRACE") == "1"
    kwargs = {}
    if trace:
        kwargs = {"trace": True, "trace_cores": list(range(N_CORES))}
    res = bass_utils.run_bass_kernel_spmd(
        nc, in_maps, core_ids=list(range(N_CORES)), **kwargs
    )
    last_results = res

    out_all = np.concatenate(
        [np.asarray(r["out"]).astype(np.float32) for r in res.results], axis=1
    )  # [D, 8*C]
    y = out_all[:, pos[:, 0]] * probs[:, 0] + out_all[:, pos[:, 1]] * probs[:, 1]
    return np.ascontiguousarray(y.T).reshape(B, T, D).astype(np.float32)


# revision 20
# speedup vs baseline: 1.0099x; 1.0010x over previous
"""MoE layer (top-2 routing, 16 experts) on 8 Trainium2 NeuronCores.

Strategy: expert-parallel. The gate (logits -> top-2 -> softmax) is computed
on the host as part of the dispatch/sharding step, replicating the reference's
jax ops so routing decisions match bit-for-bit. Tokens are gathered per
expert, experts are paired two-per-core (largest-with-smallest to balance
load), and each core runs the expert FFN (silu(x @ W1e) @ W2e) over its
gathered tokens with weights held resident in SBUF.

Device layout keeps tokens on the matmul free dimension throughout (x is
shipped transposed, [D, tokens]) so no on-chip transposes are needed:
  mm1: A^T[f, tok] += W1[d, f]^T-chunks (stationary) @ x^T[d, tok]
  silu on ScalarE, PSUM -> SBUF (bf16)
  mm2: y^T[d, tok] += W2[f, d]-chunks (stationary) @ silu(A^T)[f, tok]
All matmul operands are bfloat16: measured on HW it streams 1 moving
row/cycle like float32r but with ~11ns less per-instruction overhead
(f32r's 4-byte stationary load), and it halves every DMA stream, which
kills the DMA-starved stalls + HAM clock re-drops in the lead-in.
Accumulation stays in fp32 PSUM; end-to-end rel err ~3e-3.

The host then scatter-gathers the per-(token, slot) columns back and applies
the top-2 combine weights.
"""

import os

import ml_dtypes
import numpy as np

B, T, D, F, E = 4, 2048, 1024, 2048, 16
N_CORES = 8
P = 128
D_TILES = D // P   # 8
F_TILES = F // P   # 16
N_TOK = B * T      # 8192

_nc_cache = {}
last_results = None  # BassKernelResults of the most recent run (for test.py)


def _gate(x, Wg):
    """Top-2 routing. Uses the same jax ops as the reference so the discrete
    expert choice matches it bit-for-bit; falls back to float64 numpy."""
    h = np.asarray(x, dtype=np.float32).reshape(-1, D)
    try:
        import jax
        import jax.numpy as jnp

        logits = jnp.asarray(h) @ jnp.asarray(np.asarray(Wg, dtype=np.float32))
        scores, idx = jax.lax.top_k(logits, 2)
        probs = jax.nn.softmax(scores.astype(jnp.float32), axis=-1)
        return np.asarray(idx), np.asarray(probs, dtype=np.float32)
    except Exception:
        logits = h.astype(np.float64) @ np.asarray(Wg).astype(np.float64)
        idx = np.argsort(-logits, axis=1, kind="stable")[:, :2]
        s = np.take_along_axis(logits, idx, axis=1)
        s = s - s.max(axis=1, keepdims=True)
        p = np.exp(s)
        p /= p.sum(axis=1, keepdims=True)
        return idx.astype(np.int32), p.astype(np.float32)


def _supertiles(count):
    """Decompose a token-count capacity into supertile sizes: 512s plus one
    exact-fit remainder (multiple of 4, >= 64). bf16 matmuls run at full
    speed for any moving-dim size, so no rounding to 128/256 is needed.
    Order: largest first (its long mm1 hides the w2 startup stream), then
    ascending so the last supertile is as large as possible (opens the next
    expert's weight-slot prefetch window earlier)."""
    count = max(int(count), 64)
    m = -(-count // 4) * 4
    sizes = []
    while m > 512:
        sizes.append(512)
        m -= 512
    if m < 64:
        m = 64
    sizes.append(m)
    sizes.sort(reverse=True)
    return tuple(sizes[:1] + sorted(sizes[1:]))


def _build(st0, st1):
    """Build + compile the per-core SPMD program for supertile plans st0/st1."""
    import concourse.bacc as bacc
    import concourse.mybir as mybir
    import concourse.tile as tile

    C0, C1 = sum(st0), sum(st1)
    C = C0 + C1
    SMAX = max(max(st0), max(st1))
    bf16 = mybir.dt.bfloat16
    f32 = mybir.dt.float32

    nc = bacc.Bacc("TRN2", target_bir_lowering=False, debug=False)
    # Tokens arrive host-packed per (supertile, dtile-pair), each supertile
    # padded to SMAX columns, so every token DMA moves 2KB-contiguous
    # per-partition runs on BOTH the dram and SBUF side. Small descriptors
    # are the enemy twice over: each costs ~80-100ns of DMA-engine time
    # regardless of size, and during the 8-core startup crunch the token
    # stream competes with the weight stream for HBM. Layout: for each
    # supertile g (e0's then e1's), 4 groups of [P, 2, SMAX]: element
    # (p, j, s) = x^T[(2q+j)*128 + p, tok_s].
    n_st = len(st0) + len(st1)
    xtf = nc.dram_tensor("xtf", [n_st * D * SMAX], bf16,
                         kind="ExternalInput").ap()
    # weights arrive host-permuted so every chunk DMA is a contiguous copy:
    # w1[e, chunk, p, dt, f'], w2[e, chunk, p, ft, d']
    w1 = nc.dram_tensor("w1", [2, 8, P, D_TILES, F // 8], bf16,
                        kind="ExternalInput").ap()
    w2 = nc.dram_tensor("w2", [2, 8, P, F_TILES, D // 8], bf16,
                        kind="ExternalInput").ap()
    # bf16 output halves the store stream; host combines in fp32 (adds
    # <=2.6e-3 absmax-relative rounding on top of the ~3.3e-3 matmul error)
    out = nc.dram_tensor("out", [D, C], bf16, kind="ExternalOutput").ap()

    out_v = out.rearrange("(dt p) c -> p dt c", p=P)

    def xt_group(g, q):
        """[P, 2, SMAX] AP for dtile-pair group q of global supertile g."""
        off = (g * 4 + q) * 2 * P * SMAX
        seg = xtf[off: off + 2 * P * SMAX]
        return seg.rearrange("(p two s) -> p two s", p=P, two=2)

    with tile.TileContext(nc) as tc:
        with (
            tc.tile_pool(name="wpool", bufs=1) as wpool,
            tc.tile_pool(name="xpool", bufs=2) as xpool,
            tc.tile_pool(name="apool", bufs=1) as apool,
            tc.tile_pool(name="opool", bufs=6) as opool,
            tc.tile_pool(name="ps1", bufs=3, space="PSUM") as ps1p,
            tc.tile_pool(name="ps2", bufs=5, space="PSUM") as ps2p,
        ):
            # No PE warm-up block: the HAM clock ramp (~6us of busy time at
            # 1.2GHz before full speed) progresses the same whether the PE
            # runs dummy or real matmuls, so ramping on real work is free;
            # a measured warm-up variant only added its own busy time.
            # Weight SBUF layout groups chunks contiguously in the free dim so
            # Tile's range tracking sees each matmul depending only on its own
            # chunk's DMA (consumption-ordered streaming).
            W1C, W2C = 8, 8                      # w1: 2 f-tiles/chunk, w2: 1 d-tile/chunk
            F_PER = F // W1C                     # 256
            D_PER = D // W2C                     # 128
            for e, sts, base in ((0, st0, 0), (1, st1, C0)):
                # Queue split: the sync HWDGE queue carries ONLY the big weight
                # streams; outputs/silu ride the scalar engine and tokens ride
                # the gpsimd SWDGE queue, so a pending weight load (blocked on
                # the previous expert's last use of the slot) never
                # head-of-line-blocks them, and the token stream runs in
                # parallel with the weight stream instead of behind it.
                xt_first = xpool.tile([P, D_TILES, SMAX], bf16, tag="xt")
                w1_ch = [
                    wpool.tile([P, D_TILES, F_PER], bf16,
                               tag=f"w1c{i}", name=f"w1c{i}_{e}")
                    for i in range(W1C)
                ]
                w2_ch = [
                    wpool.tile([P, F_TILES, D_PER], bf16,
                               tag=f"w2c{j}", name=f"w2c{j}_{e}")
                    for j in range(W2C)
                ]
                g0 = 0 if e == 0 else len(st0)
                if e == 0:
                    # Startup-critical stream, interleaved in exact
                    # consumption order on the fast sync HWDGE queue (the
                    # gpsimd SWDGE queue takes ~5us to start issuing):
                    # quarter-chunk w1 pieces alternate with the token
                    # dtile-pair group each piece unblocks, so matmul
                    # (f0, d0) depends on only ~0.4MB.
                    for q in range(4):
                        nc.sync.dma_start(
                            w1_ch[0][:, 2 * q:2 * q + 2, :],
                            w1[e, 0, :, 2 * q:2 * q + 2, :],
                        )
                        nc.sync.dma_start(
                            xt_first[:, 2 * q:2 * q + 2, :], xt_group(g0, q)
                        )
                else:
                    # expert 1's first tokens have a pending buffer-recycle
                    # WAR; keep them off the sync/scalar queues so they can't
                    # head-of-line-block the weight stream or output DMAs.
                    nc.sync.dma_start(w1_ch[0][:], w1[e, 0])
                    for q in range(4):
                        nc.gpsimd.dma_start(
                            xt_first[:, 2 * q:2 * q + 2, :], xt_group(g0, q)
                        )
                for i in range(1, W1C):
                    nc.sync.dma_start(w1_ch[i][:], w1[e, i])
                for j in range(W2C):
                    nc.sync.dma_start(w2_ch[j][:], w2[e, j])
                off = base
                xt_next = xt_first
                for si, S in enumerate(sts):
                    xt_t = xt_next
                    at = apool.tile([P, F_TILES, SMAX], bf16, tag="at")
                    for f in range(F_TILES):
                        ps = ps1p.tile([P, SMAX], f32, tag="ps1")
                        for d in range(D_TILES):
                            nc.tensor.matmul(
                                ps[:, :S],
                                w1_ch[f // 2][:, d, (f % 2) * P:(f % 2 + 1) * P],
                                xt_t[:, d, :S],
                                start=(d == 0),
                                stop=(d == D_TILES - 1),
                            )
                        nc.scalar.activation(
                            at[:, f, :S], ps[:, :S],
                            mybir.ActivationFunctionType.Silu,
                        )
                        if f == 7 and si + 1 < len(sts):
                            # prefetch the next supertile's tokens mid-stream:
                            # enqueued here, its buffer-recycle WAR is already
                            # resolved, so it can't head-of-line-block the
                            # output DMAs behind it on this queue.
                            xt_next = xpool.tile(
                                [P, D_TILES, SMAX], bf16, tag="xt"
                            )
                            for q in range(4):
                                nc.gpsimd.dma_start(
                                    xt_next[:, 2 * q:2 * q + 2, :],
                                    xt_group(g0 + si + 1, q),
                                )
                    for d in range(D_TILES):
                        ps = ps2p.tile([P, SMAX], f32, tag="ps2")
                        for f in range(F_TILES):
                            nc.tensor.matmul(
                                ps[:, :S],
                                w2_ch[d][:, f],
                                at[:, f, :S],
                                start=(f == 0),
                                stop=(f == F_TILES - 1),
                            )
                        ot = opool.tile([P, SMAX], bf16, tag="ot")
                        nc.vector.tensor_copy(ot[:, :S], ps[:, :S])
                        nc.scalar.dma_start(out_v[:, d, off:off + S], ot[:, :S])
                    off += S
    nc.compile()
    return nc


def kernel(x, Wg, W1, W2):
    global last_results
    import concourse.bass_utils as bass_utils

    x = np.asarray(x, dtype=np.float32)
    W1 = np.asarray(W1, dtype=np.float32)
    W2 = np.asarray(W2, dtype=np.float32)

    idx, probs = _gate(x, Wg)
    h = x.reshape(-1, D)

    counts = np.bincount(idx.ravel(), minlength=E)
    order = np.argsort(-counts, kind="stable")
    pairs = [(int(order[i]), int(order[2 * N_CORES - 1 - i])) for i in range(N_CORES)]
    cap0 = int(counts[[p[0] for p in pairs]].max())
    cap1 = int(counts[[p[1] for p in pairs]].max())
    st0, st1 = _supertiles(cap0), _supertiles(cap1)
    C0, C1 = sum(st0), sum(st1)
    C = C0 + C1

    key = (st0, st1)
    nc = _nc_cache.get(key)
    if nc is None:
        nc = _build(st0, st1)
        _nc_cache[key] = nc

    bf16 = ml_dtypes.bfloat16
    pos = np.empty((N_TOK, 2), np.int64)
    in_maps = []
    for c, (e0, e1) in enumerate(pairs):
        ids = np.zeros(C, np.int64)
        for e, off in ((e0, 0), (e1, C0)):
            tok = np.nonzero((idx[:, 0] == e) | (idx[:, 1] == e))[0]
            ids[off:off + len(tok)] = tok
            first = idx[tok, 0] == e
            gcol = c * C + off + np.arange(len(tok))
            pos[tok[first], 0] = gcol[first]
            pos[tok[~first], 1] = gcol[~first]
        # permute weights to [e, chunk, p, tiles, cols] so each chunk DMA is
        # one contiguous copy (see _build)
        w1c = (
            W1[[e0, e1]]
            .reshape(2, D_TILES, P, 8, F // 8)
            .transpose(0, 3, 2, 1, 4)
        )
        w2c = (
            W2[[e0, e1]]
            .reshape(2, F_TILES, P, 8, D // 8)
            .transpose(0, 3, 2, 1, 4)
        )
        # token blocks packed per (supertile, dtile-pair) for 2KB-contiguous
        # DMA runs, each supertile padded to SMAX columns (see _build)
        SMAX = max(max(st0), max(st1))
        n_st = len(st0) + len(st1)
        xtf = np.zeros((n_st, 4, P, 2, SMAX), dtype=np.float32)
        gi = 0
        for off0, sts in ((0, st0), (C0, st1)):
            o = off0
            for S in sts:
                blk = h[ids[o:o + S]].T            # [D, S]
                xtf[gi, :, :, :, :S] = (
                    blk.reshape(4, 2, P, S).transpose(0, 2, 1, 3)
                )
                gi += 1
                o += S
        in_maps.append({
            "xtf": xtf.reshape(-1).astype(bf16),
            "w1": np.ascontiguousarray(w1c).astype(bf16),
            "w2": np.ascontiguousarray(w2c).astype(bf16),
        })

    trace = os.environ.get("MOE_TRACE") == "1"
    kwargs = {}
    if trace:
        kwargs = {"trace": True, "trace_cores": list(range(N_CORES))}
    res = bass_utils.run_bass_kernel_spmd(
        nc, in_maps, core_ids=list(range(N_CORES)), **kwargs
    )
    last_results = res

    out_all = np.concatenate(
        [np.asarray(r["out"]).astype(np.float32) for r in res.results], axis=1
    )  # [D, 8*C]
    y = out_all[:, pos[:, 0]] * probs[:, 0] + out_all[:, pos[:, 1]] * probs[:, 1]
    return np.ascontiguousarray(y.T).reshape(B, T, D).astype(np.float32)
